# revision 4
# baseline (speedup 1.0000x reference)
"""Distributed Trainium2 kernel for the bidirectional InfoNCE-style loss.

Math notes (vs the jax reference):
  - e1, e2 = l2norm(relu(h @ W + b)), S[i,j] = <e1_i, e2_j> / T with T=0.5.
  - The row-max subtraction in the reference cancels exactly in
    sim_pos/denom, and since <e1_i,e2_j> in [0,1], s in [0,2] -> exp is
    safe without it.  Single pass, no max.
  - Direction 2's similarity matrix is S^T: its row sums are column sums
    of the same exp'd matrix, so exp(S) is computed ONCE and reduced both
    ways.
  - log(sim_pos) = s_pos raw, so the per-row log terms only need the
    gathered positive dots and log(denom).

Sharding: rows of S (i.e. e1 / h_v1) are sharded 8 ways; e2 and W are
replicated.  Each core computes its 2048x16384 slab of exp(S), row sums
(free-dim accumulate during the Exp activation) and partial column sums
(indicator-matmul accumulated in PSUM).  The host sums partial column
sums across cores (the "all-reduce"), recomputes the 65536 positive-pair
dots from the device-computed embeddings, and assembles the scalar.
"""

import sys

sys.path.insert(0, "/opt/trn_rl_repo")

import numpy as np
import ml_dtypes

N = 16384
HID = 256
MI = 128
NCORES = 8
SHARD = N // NCORES          # 2048 rows per core
NBLK = SHARD // 128          # 16 i-blocks per core
NG = 16                      # j-groups per i-block
GW = N // NG                 # 1024 columns per group
NJT = N // 512               # 32 j-tiles (columns of 512)

_CACHE = {}
LAST_RESULT = None


def _build():
    import concourse.bacc as bacc
    import concourse.mybir as mybir
    import concourse.tile as tile

    dt = mybir.dt
    AF = mybir.ActivationFunctionType
    X = mybir.AxisListType.X

    nc = bacc.Bacc("TRN2", target_bir_lowering=False, debug=False,
                   num_devices=NCORES)

    h1t = nc.dram_tensor("h1t", [2, 128, SHARD], dt.bfloat16, kind="ExternalInput")
    h2t = nc.dram_tensor("h2t", [2, 128, N], dt.bfloat16, kind="ExternalInput")
    w = nc.dram_tensor("w", [2, 128, MI], dt.bfloat16, kind="ExternalInput")
    bb = nc.dram_tensor("bb", [MI, 1], dt.float32, kind="ExternalInput")

    e2t_out = nc.dram_tensor("e2t_out", [MI, N], dt.bfloat16, kind="ExternalOutput")
    relu1t_out = nc.dram_tensor("relu1t_out", [MI, SHARD], dt.bfloat16,
                                kind="ExternalOutput")
    inv1_out = nc.dram_tensor("inv1_out", [1, SHARD], dt.float32,
                              kind="ExternalOutput")
    rowsum_out = nc.dram_tensor("rowsum_out", [128, NBLK], dt.float32,
                                kind="ExternalOutput")
    colsum_out = nc.dram_tensor("colsum_out", [32, 512], dt.float32,
                                kind="ExternalOutput")

    with tile.TileContext(nc) as tc:
        with tc.tile_pool(name="persist", bufs=1) as per:
            e2n = per.tile([128, N], dt.bfloat16)            # normalized e2^T
            relu1_sb = per.tile([128, SHARD], dt.bfloat16)   # un-normalized relu1^T
            inv1 = per.tile([1, SHARD], dt.float32)
            scales = per.tile([128, NBLK], dt.float32)       # 2*inv1, partition-major
            racc = per.tile([128, NBLK * NG], dt.float32)    # per-(block,group) row sums
            rowsum_sb = per.tile([128, NBLK], dt.float32)
            colsum_sb = per.tile([32, 512], dt.float32)
            w_sb = per.tile([128, 2 * MI], dt.bfloat16)
            bb_sb = per.tile([128, 1], dt.float32)
            onescol = per.tile([128, 1], dt.bfloat16)
            onesrow = per.tile([1, 128], dt.float32)
            onesc = per.tile([1, 1], dt.float32)
            selwin = per.tile([128, 256], dt.bfloat16)

            nc.vector.memset(onescol[:], 1.0)
            nc.vector.memset(onesrow[:], 1.0)
            nc.vector.memset(onesc[:], 1.0)
            nc.vector.memset(selwin[:], 0.0)
            nc.vector.memset(selwin[:, 128:129], 1.0)
            nc.sync.dma_start(w_sb[:, 0:MI], w.ap()[0])
            nc.sync.dma_start(w_sb[:, MI:2 * MI], w.ap()[1])
            nc.sync.dma_start(bb_sb[:], bb.ap())

            # ---------------- preamble: projections + norms ----------------
            with tc.tile_pool(name="hin", bufs=1) as hin, \
                 tc.tile_pool(name="pre_sb", bufs=3) as pre_sb, \
                 tc.tile_pool(name="pre_ps", bufs=2, space="PSUM") as pre_ps, \
                 tc.tile_pool(name="pre_ps1", bufs=2, space="PSUM") as pre_ps1, \
                 tc.tile_pool(name="pre_ps2", bufs=1, space="PSUM") as pre_ps2:

                h2c = []
                for k in range(2):
                    for c in range(4):
                        t = hin.tile([128, 4096], dt.bfloat16, name=f"h2c_{k}_{c}")
                        nc.sync.dma_start(t[:], h2t.ap()[k, :, c * 4096:(c + 1) * 4096])
                        h2c.append((k, c, t))
                h2tile = {(k, c): t for k, c, t in h2c}
                h1sb = []
                for k in range(2):
                    t = hin.tile([128, SHARD], dt.bfloat16, name=f"h1sb_{k}")
                    nc.sync.dma_start(t[:], h1t.ap()[k])
                    h1sb.append(t)

                def proj_tile(jt, src):
                    """matmul+relu for 512 columns; returns fp32 relu tile."""
                    ps = pre_ps.tile([128, 512], dt.float32, name="proj_ps")
                    for k in range(2):
                        if src == 2:
                            rhs = h2tile[(k, jt // 8)][:, (jt % 8) * 512:(jt % 8 + 1) * 512]
                        else:
                            rhs = h1sb[k][:, jt * 512:(jt + 1) * 512]
                        nc.tensor.matmul(ps[:], w_sb[:, k * MI:(k + 1) * MI], rhs,
                                         start=(k == 0), stop=(k == 1))
                    relu_t = pre_sb.tile([128, 512], dt.float32, name="relu_t")
                    nc.scalar.activation(relu_t[:], ps[:], AF.Relu, bias=bb_sb[:])
                    return relu_t

                def inv_norm_tile(relu_t):
                    """per-column 1/||.||_2 of a [128,512] fp32 tile -> [1,512] fp32."""
                    sq = pre_sb.tile([128, 512], dt.bfloat16, name="sq_t")
                    nc.vector.tensor_mul(sq[:], relu_t[:], relu_t[:])
                    ssq = pre_ps1.tile([1, 512], dt.float32, name="ssq_ps")
                    nc.tensor.matmul(ssq[:], onescol[:], sq[:], start=True, stop=True)
                    root = pre_sb.tile([1, 512], dt.float32, name="root_t")
                    nc.scalar.activation(root[:], ssq[:], AF.Sqrt)
                    invt = pre_sb.tile([1, 512], dt.float32, name="inv_t")
                    nc.vector.reciprocal(invt[:], root[:])
                    return invt

                # e2: project, normalize columns (rank-1 broadcast matmul)
                for jt in range(NJT):
                    relu_t = proj_tile(jt, src=2)
                    invt = inv_norm_tile(relu_t)
                    bc = pre_ps1.tile([128, 512], dt.float32, name="bc_ps")
                    nc.tensor.matmul(bc[:], onesrow[:], invt[:], start=True, stop=True)
                    nc.vector.tensor_mul(e2n[:, jt * 512:(jt + 1) * 512], relu_t[:], bc[:])

                # e1 shard: project, keep un-normalized + inverse norms
                for jt in range(SHARD // 512):
                    relu_t = proj_tile(jt, src=1)
                    nc.vector.tensor_copy(relu1_sb[:, jt * 512:(jt + 1) * 512], relu_t[:])
                    invt = inv_norm_tile(relu_t)
                    nc.vector.tensor_copy(inv1[:, jt * 512:(jt + 1) * 512], invt[:])

                # scales[:, b] = 2*inv1[128b:128b+128] via transpose-matmul
                for b in range(NBLK):
                    trp = pre_ps2.tile([128, 1], dt.float32, name="tr_ps")
                    nc.tensor.matmul(trp[:], inv1[:, b * 128:(b + 1) * 128], onesc[:],
                                     start=True, stop=True)
                    nc.vector.tensor_scalar_mul(scales[:, b:b + 1], trp[:], 2.0)

            # embedding outputs (overlap with main loop)
            nc.sync.dma_start(e2t_out.ap(), e2n[:])
            nc.sync.dma_start(relu1t_out.ap(), relu1_sb[:])
            nc.sync.dma_start(inv1_out.ap(), inv1[:])

            # ---------------- main loop: exp(S), row sums, col sums ----------------
            with tc.tile_pool(name="expp", bufs=18) as expp, \
                 tc.tile_pool(name="sps", bufs=3, space="PSUM") as sps, \
                 tc.tile_pool(name="colps", bufs=1, space="PSUM") as colps:

                colacc = colps.tile([128, 512], dt.float32)

                for b in range(NBLK):
                    lhs = relu1_sb[:, b * 128:(b + 1) * 128]
                    exps = []
                    for g in range(NG):
                        s_ps = sps.tile([128, GW], dt.float32, name="s_ps")
                        for h in range(2):
                            nc.tensor.matmul(
                                s_ps[:, h * 512:(h + 1) * 512], lhs,
                                e2n[:, g * GW + h * 512:g * GW + (h + 1) * 512],
                                start=True, stop=True)
                        exp_t = expp.tile([128, GW], dt.bfloat16, name="exp_t")
                        nc.scalar.activation(exp_t[:], s_ps[:], AF.Exp,
                                             scale=scales[:, b:b + 1],
                                             accum_out=racc[:, b * NG + g:b * NG + g + 1])
                        exps.append(exp_t)
                    for t in range(NJT):
                        g, h = t // 2, t % 2
                        nc.tensor.matmul(
                            colacc[:], selwin[:, 128 - t:256 - t],
                            exps[g][:, h * 512:(h + 1) * 512],
                            start=(b == 0 and t == 0),
                            stop=(b == NBLK - 1 and t == NJT - 1))

                nc.vector.tensor_copy(colsum_sb[:], colacc[0:32, :])

            for b in range(NBLK):
                nc.vector.reduce_sum(rowsum_sb[:, b:b + 1],
                                     racc[:, b * NG:(b + 1) * NG], axis=X)
            nc.sync.dma_start(rowsum_out.ap(), rowsum_sb[:])
            nc.sync.dma_start(colsum_out.ap(), colsum_sb[:])

    nc.compile()
    return nc


def _get_nc():
    if "nc" not in _CACHE:
        _CACHE["nc"] = _build()
    return _CACHE["nc"]


def kernel(h_v1, h_v2, W, b, pos_row, pos_col):
    global LAST_RESULT
    from concourse import bass_utils

    bf16 = ml_dtypes.bfloat16
    h2t = np.ascontiguousarray(np.asarray(h_v2, np.float32).T).astype(bf16)
    h2t = h2t.reshape(2, 128, N)
    wct = np.asarray(W, np.float32).astype(bf16).reshape(2, 128, MI)
    bbc = np.asarray(b, np.float32).reshape(MI, 1)

    in_maps = []
    for c in range(NCORES):
        sh = np.ascontiguousarray(
            np.asarray(h_v1[c * SHARD:(c + 1) * SHARD], np.float32).T
        ).astype(bf16).reshape(2, 128, SHARD)
        in_maps.append({"h1t": sh, "h2t": h2t, "w": wct, "bb": bbc})

    nc = _get_nc()
    res = bass_utils.run_bass_kernel_spmd(nc, in_maps, core_ids=list(range(NCORES)))
    LAST_RESULT = res
    rs = res.results

    colsum = np.zeros(N, np.float64)
    for r in rs:
        colsum += r["colsum_out"].reshape(-1).astype(np.float64)
    rowsum = np.concatenate(
        [r["rowsum_out"].T.reshape(-1) for r in rs]).astype(np.float64)

    e2nr = rs[0]["e2t_out"].astype(np.float32).T           # [N, 128] normalized
    e1nr = np.concatenate(
        [(r["relu1t_out"].astype(np.float32) * r["inv1_out"].reshape(1, -1)).T
         for r in rs], axis=0)                              # [N, 128] normalized

    pr = np.asarray(pos_row).astype(np.int64)
    pc = np.asarray(pos_col).astype(np.int64)
    s1 = 2.0 * np.einsum("kf,kf->k", e1nr[pr], e2nr[pc], optimize=True)
    s2 = 2.0 * np.einsum("kf,kf->k", e1nr[pc], e2nr[pr], optimize=True)

    cnt = np.bincount(pr, minlength=N).astype(np.float64)
    B1 = np.bincount(pr, weights=np.exp(s1), minlength=N)
    A1 = np.bincount(pr, weights=s1, minlength=N)
    B2 = np.bincount(pr, weights=np.exp(s2), minlength=N)
    A2 = np.bincount(pr, weights=s2, minlength=N)

    per1 = (A1 - cnt * np.log(rowsum - B1)) / cnt
    per2 = (A2 - cnt * np.log(colsum - B2)) / cnt
    loss = -0.5 * (per1.mean() + per2.mean())
    return np.array(loss, dtype=np.float32)


# revision 8
# speedup vs baseline: 1.2027x; 1.2027x over previous
"""Distributed Trainium2 kernel for the bidirectional InfoNCE-style loss.

Math notes (vs the jax reference):
  - e1, e2 = l2norm(relu(h @ W + b)), S[i,j] = <e1_i, e2_j> / T with T=0.5.
  - The row-max subtraction in the reference cancels exactly in
    sim_pos/denom, and since <e1_i,e2_j> in [0,1], s in [0,2] -> exp is
    safe without it.  Single pass, no max.
  - Direction 2's similarity matrix is S^T: its row sums are column sums
    of the same exp'd matrix, so exp(S) is computed ONCE and reduced both
    ways.
  - log(sim_pos) = s_pos raw, so the per-row log terms only need the
    gathered positive dots and log(denom).

Sharding: rows of S (i.e. e1 / h_v1) are sharded 8 ways; e2 and W are
replicated.  Each core computes its 2048x16384 slab of exp(S), row sums
(free-dim accumulate during the Exp activation) and partial column sums
(indicator-matmul accumulated in PSUM).  The host sums partial column
sums across cores (the "all-reduce"), recomputes the 65536 positive-pair
dots from the device-computed embeddings, and assembles the scalar.
"""

import sys

sys.path.insert(0, "/opt/trn_rl_repo")

import numpy as np
import ml_dtypes

N = 16384
HID = 256
MI = 128
NCORES = 8
SHARD = N // NCORES          # 2048 rows per core
NBLK = SHARD // 128          # 16 i-blocks per core
NG = 8                       # j-groups per i-block
GW = N // NG                 # 2048 columns per group
NJT = N // 512               # 32 j-tiles (columns of 512)

_CACHE = {}
LAST_RESULT = None


def _build():
    import concourse.bacc as bacc
    import concourse.mybir as mybir
    import concourse.tile as tile

    dt = mybir.dt
    AF = mybir.ActivationFunctionType
    X = mybir.AxisListType.X

    nc = bacc.Bacc("TRN2", target_bir_lowering=False, debug=False,
                   num_devices=NCORES)

    h1t = nc.dram_tensor("h1t", [2, 128, SHARD], dt.bfloat16, kind="ExternalInput")
    h2t = nc.dram_tensor("h2t", [2, 128, N], dt.bfloat16, kind="ExternalInput")
    w = nc.dram_tensor("w", [2, 128, MI], dt.bfloat16, kind="ExternalInput")
    bb = nc.dram_tensor("bb", [MI, 1], dt.float32, kind="ExternalInput")

    e2t_out = nc.dram_tensor("e2t_out", [MI, N], dt.bfloat16, kind="ExternalOutput")
    relu1t_out = nc.dram_tensor("relu1t_out", [MI, SHARD], dt.bfloat16,
                                kind="ExternalOutput")
    inv1_out = nc.dram_tensor("inv1_out", [1, SHARD], dt.float32,
                              kind="ExternalOutput")
    rowsum_out = nc.dram_tensor("rowsum_out", [128, NBLK], dt.float32,
                                kind="ExternalOutput")
    colsum_out = nc.dram_tensor("colsum_out", [32, 512], dt.float32,
                                kind="ExternalOutput")

    with tile.TileContext(nc) as tc:
        with tc.tile_pool(name="persist", bufs=1) as per:
            e2n = per.tile([128, N], dt.bfloat16)            # normalized e2^T
            colacc = per.tile([128, N], dt.bfloat16)         # per-partition col sums
            relu1_sb = per.tile([128, SHARD], dt.bfloat16)   # un-normalized relu1^T
            inv1 = per.tile([1, SHARD], dt.float32)
            scales = per.tile([128, NBLK], dt.float32)       # 2*inv1, partition-major
            racc = per.tile([128, NBLK * NG], dt.float32)    # per-(block,group) row sums
            rowsum_sb = per.tile([128, NBLK], dt.float32)
            colsum_sb = per.tile([32, 512], dt.float32)
            w_sb = per.tile([128, 2 * MI], dt.bfloat16)
            bb_sb = per.tile([128, 1], dt.float32)
            onescol = per.tile([128, 1], dt.bfloat16)
            onesrow = per.tile([1, 128], dt.float32)
            onesc = per.tile([1, 1], dt.float32)
            selwin = per.tile([128, 256], dt.bfloat16)

            nc.vector.memset(onescol[:], 1.0)
            nc.vector.memset(onesrow[:], 1.0)
            nc.vector.memset(onesc[:], 1.0)
            nc.vector.memset(selwin[:], 0.0)
            nc.vector.memset(selwin[:, 128:129], 1.0)
            nc.sync.dma_start(w_sb[:, 0:MI], w.ap()[0])
            nc.sync.dma_start(w_sb[:, MI:2 * MI], w.ap()[1])
            nc.sync.dma_start(bb_sb[:], bb.ap())

            # ---------------- preamble: projections + norms ----------------
            with tc.tile_pool(name="hin", bufs=1) as hin, \
                 tc.tile_pool(name="pre_sb", bufs=3) as pre_sb, \
                 tc.tile_pool(name="pre_ps", bufs=2, space="PSUM") as pre_ps, \
                 tc.tile_pool(name="pre_ps1", bufs=2, space="PSUM") as pre_ps1, \
                 tc.tile_pool(name="pre_ps2", bufs=1, space="PSUM") as pre_ps2:

                h2c = []
                for k in range(2):
                    for c in range(4):
                        t = hin.tile([128, 4096], dt.bfloat16, name=f"h2c_{k}_{c}")
                        nc.sync.dma_start(t[:], h2t.ap()[k, :, c * 4096:(c + 1) * 4096])
                        h2c.append((k, c, t))
                h2tile = {(k, c): t for k, c, t in h2c}
                h1sb = []
                for k in range(2):
                    t = hin.tile([128, SHARD], dt.bfloat16, name=f"h1sb_{k}")
                    nc.sync.dma_start(t[:], h1t.ap()[k])
                    h1sb.append(t)

                def proj_tile(jt, src):
                    """matmul+relu for 512 columns; returns fp32 relu tile."""
                    ps = pre_ps.tile([128, 512], dt.float32, name="proj_ps")
                    for k in range(2):
                        if src == 2:
                            rhs = h2tile[(k, jt // 8)][:, (jt % 8) * 512:(jt % 8 + 1) * 512]
                        else:
                            rhs = h1sb[k][:, jt * 512:(jt + 1) * 512]
                        nc.tensor.matmul(ps[:], w_sb[:, k * MI:(k + 1) * MI], rhs,
                                         start=(k == 0), stop=(k == 1))
                    relu_t = pre_sb.tile([128, 512], dt.float32, name="relu_t")
                    # relu(x + b) = max(x + b, 0) on DVE (keeps ScalarE free)
                    nc.vector.tensor_scalar(relu_t[:], ps[:], bb_sb[:], 0.0,
                                            mybir.AluOpType.add, mybir.AluOpType.max)
                    return relu_t

                def inv_norm_tile(relu_t):
                    """per-column 1/||.||_2 of a [128,512] fp32 tile -> [1,512] fp32."""
                    sq = pre_sb.tile([128, 512], dt.bfloat16, name="sq_t")
                    nc.vector.tensor_mul(sq[:], relu_t[:], relu_t[:])
                    ssq = pre_ps1.tile([1, 512], dt.float32, name="ssq_ps")
                    nc.tensor.matmul(ssq[:], onescol[:], sq[:], start=True, stop=True)
                    root = pre_sb.tile([1, 512], dt.float32, name="root_t")
                    nc.scalar.activation(root[:], ssq[:], AF.Sqrt)
                    invt = pre_sb.tile([1, 512], dt.float32, name="inv_t")
                    nc.vector.reciprocal_approx_fast(invt[:], root[:])
                    return invt

                # e2: project, normalize columns (rank-1 broadcast matmul)
                for jt in range(NJT):
                    relu_t = proj_tile(jt, src=2)
                    invt = inv_norm_tile(relu_t)
                    bc = pre_ps1.tile([128, 512], dt.float32, name="bc_ps")
                    nc.tensor.matmul(bc[:], onesrow[:], invt[:], start=True, stop=True)
                    nc.vector.tensor_mul(e2n[:, jt * 512:(jt + 1) * 512], relu_t[:], bc[:])

                # e1 shard: project, keep un-normalized + inverse norms
                for jt in range(SHARD // 512):
                    relu_t = proj_tile(jt, src=1)
                    nc.vector.tensor_copy(relu1_sb[:, jt * 512:(jt + 1) * 512], relu_t[:])
                    invt = inv_norm_tile(relu_t)
                    nc.vector.tensor_copy(inv1[:, jt * 512:(jt + 1) * 512], invt[:])

                # scales[:, b] = 2*inv1[128b:128b+128] via transpose-matmul
                for b in range(NBLK):
                    trp = pre_ps2.tile([128, 1], dt.float32, name="tr_ps")
                    nc.tensor.matmul(trp[:], inv1[:, b * 128:(b + 1) * 128], onesc[:],
                                     start=True, stop=True)
                    nc.vector.tensor_scalar_mul(scales[:, b:b + 1], trp[:], 2.0)

            # embedding outputs (overlap with main loop)
            nc.sync.dma_start(e2t_out.ap(), e2n[:])
            nc.sync.dma_start(relu1t_out.ap(), relu1_sb[:])
            nc.sync.dma_start(inv1_out.ap(), inv1[:])

            # ---------------- main loop: exp(S), row sums, col sums ----------------
            with tc.tile_pool(name="expp", bufs=4) as expp, \
                 tc.tile_pool(name="sps", bufs=2, space="PSUM") as sps:

                for b in range(NBLK):
                    lhs = relu1_sb[:, b * 128:(b + 1) * 128]
                    for g in range(NG):
                        s_ps = sps.tile([128, GW], dt.float32, name="s_ps")
                        for h in range(4):
                            nc.tensor.matmul(
                                s_ps[:, h * 512:(h + 1) * 512], lhs,
                                e2n[:, g * GW + h * 512:g * GW + (h + 1) * 512],
                                start=True, stop=True)
                        exp_t = expp.tile([128, GW], dt.bfloat16, name="exp_t")
                        nc.scalar.activation(exp_t[:], s_ps[:], AF.Exp,
                                             scale=scales[:, b:b + 1],
                                             accum_out=racc[:, b * NG + g:b * NG + g + 1])
                        # col-sum accumulation per partition (bf16 TT -> 2x mode)
                        cslice = colacc[:, g * GW:(g + 1) * GW]
                        if b == 0:
                            nc.vector.tensor_copy(cslice, exp_t[:])
                        else:
                            nc.vector.tensor_add(cslice, cslice, exp_t[:])

            # partition-reduce colacc: row t of colacc_ps = colsum[512t:512t+512]
            with tc.tile_pool(name="colps", bufs=1, space="PSUM") as colps:
                colacc_ps = colps.tile([128, 512], dt.float32)
                for t in range(NJT):
                    nc.tensor.matmul(
                        colacc_ps[:], selwin[:, 128 - t:256 - t],
                        colacc[:, t * 512:(t + 1) * 512],
                        start=(t == 0), stop=(t == NJT - 1))
                nc.vector.tensor_copy(colsum_sb[:], colacc_ps[0:32, :])

            for b in range(NBLK):
                nc.vector.reduce_sum(rowsum_sb[:, b:b + 1],
                                     racc[:, b * NG:(b + 1) * NG], axis=X)
            nc.sync.dma_start(rowsum_out.ap(), rowsum_sb[:])
            nc.sync.dma_start(colsum_out.ap(), colsum_sb[:])

    nc.compile()
    return nc


def _get_nc():
    if "nc" not in _CACHE:
        _CACHE["nc"] = _build()
    return _CACHE["nc"]


def kernel(h_v1, h_v2, W, b, pos_row, pos_col):
    global LAST_RESULT
    from concourse import bass_utils

    bf16 = ml_dtypes.bfloat16
    h2t = np.ascontiguousarray(np.asarray(h_v2, np.float32).T).astype(bf16)
    h2t = h2t.reshape(2, 128, N)
    wct = np.asarray(W, np.float32).astype(bf16).reshape(2, 128, MI)
    bbc = np.asarray(b, np.float32).reshape(MI, 1)

    in_maps = []
    for c in range(NCORES):
        sh = np.ascontiguousarray(
            np.asarray(h_v1[c * SHARD:(c + 1) * SHARD], np.float32).T
        ).astype(bf16).reshape(2, 128, SHARD)
        in_maps.append({"h1t": sh, "h2t": h2t, "w": wct, "bb": bbc})

    nc = _get_nc()
    res = bass_utils.run_bass_kernel_spmd(nc, in_maps, core_ids=list(range(NCORES)))
    LAST_RESULT = res
    rs = res.results

    colsum = np.zeros(N, np.float64)
    for r in rs:
        colsum += r["colsum_out"].reshape(-1).astype(np.float64)
    rowsum = np.concatenate(
        [r["rowsum_out"].T.reshape(-1) for r in rs]).astype(np.float64)

    e2nr = rs[0]["e2t_out"].astype(np.float32).T           # [N, 128] normalized
    e1nr = np.concatenate(
        [(r["relu1t_out"].astype(np.float32) * r["inv1_out"].reshape(1, -1)).T
         for r in rs], axis=0)                              # [N, 128] normalized

    pr = np.asarray(pos_row).astype(np.int64)
    pc = np.asarray(pos_col).astype(np.int64)
    s1 = 2.0 * np.einsum("kf,kf->k", e1nr[pr], e2nr[pc], optimize=True)
    s2 = 2.0 * np.einsum("kf,kf->k", e1nr[pc], e2nr[pr], optimize=True)

    cnt = np.bincount(pr, minlength=N).astype(np.float64)
    B1 = np.bincount(pr, weights=np.exp(s1), minlength=N)
    A1 = np.bincount(pr, weights=s1, minlength=N)
    B2 = np.bincount(pr, weights=np.exp(s2), minlength=N)
    A2 = np.bincount(pr, weights=s2, minlength=N)

    per1 = (A1 - cnt * np.log(rowsum - B1)) / cnt
    per2 = (A2 - cnt * np.log(colsum - B2)) / cnt
    loss = -0.5 * (per1.mean() + per2.mean())
    return np.array(loss, dtype=np.float32)


# revision 15
# speedup vs baseline: 1.2963x; 1.0778x over previous
"""Distributed Trainium2 kernel for the bidirectional InfoNCE-style loss.

Math notes (vs the jax reference):
  - e1, e2 = l2norm(relu(h @ W + b)), S[i,j] = <e1_i, e2_j> / T with T=0.5.
  - The row-max subtraction in the reference cancels exactly in
    sim_pos/denom, and since <e1_i,e2_j> in [0,1], s in [0,2] -> exp is
    safe without it.  Single pass, no max.
  - Direction 2's similarity matrix is S^T: its row sums are column sums
    of the same exp'd matrix, so exp(S) is computed ONCE and reduced both
    ways.
  - log(sim_pos) = s_pos raw, so the per-row log terms only need the
    gathered positive dots and log(denom).

Sharding: rows of S (i.e. e1 / h_v1) are sharded 8 ways; e2 and W are
replicated.  Each core computes its 2048x16384 slab of exp(S), row sums
(free-dim accumulate during the Exp activation) and partial column sums
(indicator-matmul accumulated in PSUM).  The host sums partial column
sums across cores (the "all-reduce"), recomputes the 65536 positive-pair
dots from the device-computed embeddings, and assembles the scalar.
"""

import sys

sys.path.insert(0, "/opt/trn_rl_repo")

import numpy as np
import ml_dtypes

N = 16384
HID = 256
MI = 128
NCORES = 8
SHARD = N // NCORES          # 2048 rows per core
NBLK = SHARD // 128          # 16 i-blocks per core
NG = 8                       # j-groups per i-block
GW = N // NG                 # 2048 columns per group
NJT = N // 512               # 32 j-tiles (columns of 512)

_CACHE = {}
LAST_RESULT = None


def _build():
    import concourse.bacc as bacc
    import concourse.mybir as mybir
    import concourse.tile as tile

    dt = mybir.dt
    AF = mybir.ActivationFunctionType
    X = mybir.AxisListType.X

    nc = bacc.Bacc("TRN2", target_bir_lowering=False, debug=False,
                   num_devices=NCORES)

    h1t = nc.dram_tensor("h1t", [2, 128, SHARD], dt.bfloat16, kind="ExternalInput")
    h2t = nc.dram_tensor("h2t", [2, 128, N], dt.bfloat16, kind="ExternalInput")
    w = nc.dram_tensor("w", [2, 128, MI], dt.bfloat16, kind="ExternalInput")
    bb = nc.dram_tensor("bb", [MI, 1], dt.float32, kind="ExternalInput")

    e2t_out = nc.dram_tensor("e2t_out", [MI, N], dt.bfloat16, kind="ExternalOutput")
    relu1t_out = nc.dram_tensor("relu1t_out", [MI, SHARD], dt.bfloat16,
                                kind="ExternalOutput")
    inv1_out = nc.dram_tensor("inv1_out", [1, SHARD], dt.float32,
                              kind="ExternalOutput")
    rowsum_out = nc.dram_tensor("rowsum_out", [128, NBLK], dt.float32,
                                kind="ExternalOutput")
    colsum_out = nc.dram_tensor("colsum_out", [32, 512], dt.float32,
                                kind="ExternalOutput")

    with tile.TileContext(nc) as tc:
        with tc.tile_pool(name="persist", bufs=1) as per:
            # per-group tiles so the main loop can start before the whole
            # preamble finishes (dep tracking is per-tile)
            e2ng = [per.tile([128, GW], dt.bfloat16, name=f"e2n_{g}")
                    for g in range(NG)]                      # normalized e2^T
            relu2g = [per.tile([128, GW], dt.bfloat16, name=f"relu2_{g}")
                      for g in range(NG)]                    # un-normalized relu2^T
            colaccg = [per.tile([128, GW], dt.bfloat16, name=f"colacc_{g}")
                       for g in range(NG)]                   # per-partition col sums
            relu1_sb = per.tile([128, SHARD], dt.bfloat16)   # un-normalized relu1^T
            inv1 = per.tile([1, SHARD], dt.float32)
            scales = per.tile([128, NBLK], dt.float32)       # 2*inv1, partition-major
            racc = per.tile([128, NBLK * NG], dt.float32)    # per-(block,group) row sums
            rowsum_sb = per.tile([128, NBLK], dt.float32)
            colsum_sb = per.tile([32, 512], dt.float32)
            w_sb = per.tile([128, 2 * MI], dt.bfloat16)
            bb_sb = per.tile([128, 1], dt.float32)
            onescol = per.tile([128, 1], dt.bfloat16)
            onesrow = per.tile([1, 128], dt.float32)
            onesc = per.tile([1, 1], dt.float32)
            selwin = per.tile([128, 256], dt.bfloat16)

            nc.vector.memset(onescol[:], 1.0)
            nc.vector.memset(onesrow[:], 1.0)
            nc.vector.memset(onesc[:], 1.0)
            nc.vector.memset(selwin[:], 0.0)
            nc.vector.memset(selwin[:, 128:129], 1.0)
            nc.sync.dma_start(w_sb[:, 0:MI], w.ap()[0])
            nc.sync.dma_start(w_sb[:, MI:2 * MI], w.ap()[1])
            nc.sync.dma_start(bb_sb[:], bb.ap())

            # ---------------- preamble: projections + norms ----------------
            with tc.tile_pool(name="hin", bufs=1) as hin, \
                 tc.tile_pool(name="pre_sb", bufs=4) as pre_sb, \
                 tc.tile_pool(name="pre_row", bufs=3) as pre_row, \
                 tc.tile_pool(name="pre_ps", bufs=3, space="PSUM") as pre_ps, \
                 tc.tile_pool(name="pre_ps1", bufs=2, space="PSUM") as pre_ps1, \
                 tc.tile_pool(name="pre_ps2", bufs=1, space="PSUM") as pre_ps2:

                h1sb = []
                for k in range(2):
                    t = hin.tile([128, SHARD], dt.bfloat16, name=f"h1sb_{k}")
                    nc.sync.dma_start(t[:], h1t.ap()[k])
                    h1sb.append(t)
                # chunk pairs share 2 rotating slots per k (tag = name)
                h2tile = {}
                for c in range(4):
                    for k in range(2):
                        t = hin.tile([128, 4096], dt.bfloat16, name=f"h2c_{k}_{c % 2}")
                        nc.sync.dma_start(t[:], h2t.ap()[k, :, c * 4096:(c + 1) * 4096])
                        h2tile[(k, c)] = t

                def proj_tile(jt, src, out_bf, out_slice):
                    """matmul + relu(x+b) for 512 cols -> bf16 slice of out_bf."""
                    ps = pre_ps.tile([128, 512], dt.float32, name="proj_ps")
                    for k in range(2):
                        if src == 2:
                            rhs = h2tile[(k, jt // 8)][:, (jt % 8) * 512:(jt % 8 + 1) * 512]
                        else:
                            rhs = h1sb[k][:, jt * 512:(jt + 1) * 512]
                        nc.tensor.matmul(ps[:], w_sb[:, k * MI:(k + 1) * MI], rhs,
                                         start=(k == 0), stop=(k == 1))
                    # relu(x + b) = max(x + b, 0) on DVE (keeps ScalarE free)
                    nc.vector.tensor_scalar(out_bf[:, out_slice], ps[:], bb_sb[:], 0.0,
                                            mybir.AluOpType.add, mybir.AluOpType.max)

                def inv_norm_tile(relu_bf, invt_out):
                    """per-column 1/||.||_2 of a [128,512] bf16 slice -> [1,512] fp32."""
                    sq = pre_sb.tile([128, 512], dt.bfloat16, name="sq_t")
                    nc.vector.tensor_mul(sq[:], relu_bf, relu_bf)
                    ssq = pre_ps1.tile([1, 512], dt.float32, name="ssq_ps")
                    nc.tensor.matmul(ssq[:], onescol[:], sq[:], start=True, stop=True)
                    root = pre_row.tile([1, 512], dt.float32, name="root_t")
                    nc.scalar.activation(root[:], ssq[:], AF.Sqrt)
                    nc.vector.reciprocal_approx_fast(invt_out, root[:])

                # e1 shard first: small, unblocks scales + relu1 for the main loop
                for jt in range(SHARD // 512):
                    proj_tile(jt, 1, relu1_sb, slice(jt * 512, (jt + 1) * 512))
                for jt in range(SHARD // 512):
                    inv_norm_tile(relu1_sb[:, jt * 512:(jt + 1) * 512],
                                  inv1[:, jt * 512:(jt + 1) * 512])
                # scales[:, b] = 2*inv1[128b:128b+128] via transpose-matmul
                for b in range(NBLK):
                    trp = pre_ps2.tile([128, 1], dt.float32, name="tr_ps")
                    nc.tensor.matmul(trp[:], inv1[:, b * 128:(b + 1) * 128], onesc[:],
                                     start=True, stop=True)
                    nc.vector.tensor_scalar_mul(scales[:, b:b + 1], trp[:], 2.0)

                # e2 pass 1: project+relu all tiles (dense per-engine streams)
                for jt in range(NJT):
                    proj_tile(jt, 2, relu2g[jt // 4], slice((jt % 4) * 512, (jt % 4 + 1) * 512))
                # e2 pass 2: norms + column normalize (rank-1 broadcast matmul)
                for jt in range(NJT):
                    rslice = relu2g[jt // 4][:, (jt % 4) * 512:(jt % 4 + 1) * 512]
                    invt = pre_row.tile([1, 512], dt.float32, name="inv2_t")
                    inv_norm_tile(rslice, invt[:])
                    bc = pre_ps1.tile([128, 512], dt.float32, name="bc_ps")
                    nc.tensor.matmul(bc[:], onesrow[:], invt[:], start=True, stop=True)
                    nc.vector.tensor_mul(
                        e2ng[jt // 4][:, (jt % 4) * 512:(jt % 4 + 1) * 512], rslice, bc[:])

            # embedding outputs (overlap with main loop)
            for g in range(NG):
                nc.sync.dma_start(e2t_out.ap()[:, g * GW:(g + 1) * GW], e2ng[g][:])
            nc.sync.dma_start(relu1t_out.ap(), relu1_sb[:])
            nc.sync.dma_start(inv1_out.ap(), inv1[:])

            # ---------------- main loop: exp(S), row sums, col sums ----------------
            with tc.tile_pool(name="expp", bufs=4) as expp, \
                 tc.tile_pool(name="sps", bufs=2, space="PSUM") as sps:

                for b in range(NBLK):
                    lhs = relu1_sb[:, b * 128:(b + 1) * 128]
                    for g in range(NG):
                        s_ps = sps.tile([128, GW], dt.float32, name="s_ps")
                        for h in range(4):
                            nc.tensor.matmul(
                                s_ps[:, h * 512:(h + 1) * 512], lhs,
                                e2ng[g][:, h * 512:(h + 1) * 512],
                                start=True, stop=True)
                        exp_t = expp.tile([128, GW], dt.bfloat16, name="exp_t")
                        nc.scalar.activation(exp_t[:], s_ps[:], AF.Exp,
                                             scale=scales[:, b:b + 1],
                                             accum_out=racc[:, b * NG + g:b * NG + g + 1])
                        # col-sum accumulation per partition (bf16 TT -> 2x mode)
                        cslice = colaccg[g][:]
                        if b == 0:
                            nc.vector.tensor_copy(cslice, exp_t[:])
                        else:
                            nc.vector.tensor_add(cslice, cslice, exp_t[:])

            # partition-reduce colacc: row t of colacc_ps = colsum[512t:512t+512]
            with tc.tile_pool(name="colps", bufs=1, space="PSUM") as colps:
                colacc_ps = colps.tile([128, 512], dt.float32)
                for t in range(NJT):
                    nc.tensor.matmul(
                        colacc_ps[:], selwin[:, 128 - t:256 - t],
                        colaccg[t // 4][:, (t % 4) * 512:(t % 4 + 1) * 512],
                        start=(t == 0), stop=(t == NJT - 1))
                nc.vector.tensor_copy(colsum_sb[:], colacc_ps[0:32, :])

            for b in range(NBLK):
                nc.vector.reduce_sum(rowsum_sb[:, b:b + 1],
                                     racc[:, b * NG:(b + 1) * NG], axis=X)
            nc.sync.dma_start(rowsum_out.ap(), rowsum_sb[:])
            nc.sync.dma_start(colsum_out.ap(), colsum_sb[:])

    nc.compile()
    return nc


def _get_nc():
    if "nc" not in _CACHE:
        _CACHE["nc"] = _build()
    return _CACHE["nc"]


def kernel(h_v1, h_v2, W, b, pos_row, pos_col):
    global LAST_RESULT
    from concourse import bass_utils

    bf16 = ml_dtypes.bfloat16
    h2t = np.ascontiguousarray(np.asarray(h_v2, np.float32).T).astype(bf16)
    h2t = h2t.reshape(2, 128, N)
    wct = np.asarray(W, np.float32).astype(bf16).reshape(2, 128, MI)
    bbc = np.asarray(b, np.float32).reshape(MI, 1)

    in_maps = []
    for c in range(NCORES):
        sh = np.ascontiguousarray(
            np.asarray(h_v1[c * SHARD:(c + 1) * SHARD], np.float32).T
        ).astype(bf16).reshape(2, 128, SHARD)
        in_maps.append({"h1t": sh, "h2t": h2t, "w": wct, "bb": bbc})

    nc = _get_nc()
    res = bass_utils.run_bass_kernel_spmd(nc, in_maps, core_ids=list(range(NCORES)))
    LAST_RESULT = res
    rs = res.results

    colsum = np.zeros(N, np.float64)
    for r in rs:
        colsum += r["colsum_out"].reshape(-1).astype(np.float64)
    rowsum = np.concatenate(
        [r["rowsum_out"].T.reshape(-1) for r in rs]).astype(np.float64)

    e2nr = rs[0]["e2t_out"].astype(np.float32).T           # [N, 128] normalized
    e1nr = np.concatenate(
        [(r["relu1t_out"].astype(np.float32) * r["inv1_out"].reshape(1, -1)).T
         for r in rs], axis=0)                              # [N, 128] normalized

    pr = np.asarray(pos_row).astype(np.int64)
    pc = np.asarray(pos_col).astype(np.int64)
    s1 = 2.0 * np.einsum("kf,kf->k", e1nr[pr], e2nr[pc], optimize=True)
    s2 = 2.0 * np.einsum("kf,kf->k", e1nr[pc], e2nr[pr], optimize=True)

    cnt = np.bincount(pr, minlength=N).astype(np.float64)
    B1 = np.bincount(pr, weights=np.exp(s1), minlength=N)
    A1 = np.bincount(pr, weights=s1, minlength=N)
    B2 = np.bincount(pr, weights=np.exp(s2), minlength=N)
    A2 = np.bincount(pr, weights=s2, minlength=N)

    per1 = (A1 - cnt * np.log(rowsum - B1)) / cnt
    per2 = (A2 - cnt * np.log(colsum - B2)) / cnt
    loss = -0.5 * (per1.mean() + per2.mean())
    return np.array(loss, dtype=np.float32)
